# revision 6
# baseline (speedup 1.0000x reference)
"""Trainium2 Bass kernel for a single-step attention decoder (DecoderRNN).

Computation (batch=1, single decode step):
  embedded = emb[word]                                  [E]
  x  = concat(last_context, embedded)                   [H+E]
  gates = W_ih @ x + b_ih + W_hh @ h0 + b_hh            [4H] (i,f,g,o)
  c  = sig(f)*c0 + sig(i)*tanh(g);  ht = sig(o)*tanh(c) [H]
  scores = tanh(enc @ Wa_e.T + (Wa_h @ ht + b_attn)) @ v  [S]
  w  = softmax(scores);  context = w @ enc              [E]
  ht_tilda = tanh(W_ah @ concat(context, ht) + b_ah)    [H]
  out = log_softmax(W_out @ ht_tilda + b_out)           [V]

Sharding across 8 NeuronCores (tensor parallel):
  - LSTM / attn-linear / W_ah rows: hidden dim sharded (128 rows per core)
  - encoder_out: sequence-sharded (256 steps per core) for the score matmul
  - W_out: vocab-sharded (4000 rows per core, padded to 4096)
  Tiny collectives stitch the stages: AllGather(ht), AllGather(u),
  AllReduce(unnormalized context + softmax normalizer), AllGather(ht_tilda),
  AllGather(per-core logsumexp partials).

Everything heavy is HBM-bandwidth-bound weight streaming (~28 MB/core fp32);
matvecs run on the PE with host-pre-transposed weights, softmax/log-softmax
normalizers via on-device exp + partition-sum matmuls (no max subtraction
needed: logits/scores are O(1) by construction).
"""
import numpy as np

import concourse.bass as bass
import concourse.mybir as mybir
import concourse.tile as tile
from concourse import bacc, bass_utils
from concourse.bass_interp import get_hw_module

NCORES = 8
V, E, H, S = 32000, 1024, 1024, 2048
HC = H // NCORES      # 128  hidden chunk per core
SC = S // NCORES      # 256  sequence chunk per core
VC = V // NCORES      # 4000 vocab rows per core
VP = 4096             # padded vocab shard (32 tiles of 128)
KX = (E + H) // 128   # 16   contraction chunks for x / cat
KH = H // 128         # 8    contraction chunks for ht-sized vectors
NEG_BIG = -1.0e5      # pad-row bias: exp() underflows to exactly 0

f32 = mybir.dt.float32
AF = mybir.ActivationFunctionType
ALU = mybir.AluOpType


# ---------------------------------------------------------------- device code
def _emit_iter(nc, tc, I, O, pools):
    """Emit one full decoder step. I/O are dicts of DRAM APs."""
    const, wihp, wathp, waep, enctp, encnp, tzp, wahp, wop, smp, dram, ps = pools

    def dma(dst, src):
        nc.sync.dma_start(dst, src)

    ones = pools_ones[0]  # [128,1] of 1.0, set up once outside

    # ---- small constants ----
    xk = smp.tile([128, KX], f32, tag="xk")
    dma(xk[:], I["xk"])
    hb = smp.tile([128, 4], f32, tag="hb")
    dma(hb[:], I["hbias"])
    c0 = smp.tile([128, 1], f32, tag="c0")
    dma(c0[:], I["c0k"])
    battn = smp.tile([128, 1], f32, tag="battn")
    dma(battn[:], I["battn"])
    bah = smp.tile([128, 1], f32, tag="bah")
    dma(bah[:], I["bah"])
    vk = smp.tile([128, KH], f32, tag="vk")
    dma(vk[:], I["vk"])
    bout = smp.tile([128, 32], f32, tag="bout")
    dma(bout[:], I["bout"])

    # ================= Stage A: LSTM gates -> ht_k, c_k =================
    wih = []
    for kc in range(KX):
        t = wihp.tile([128, 512], f32, tag=f"wih{kc % 8}")
        dma(t[:], I["wihT"][kc * 128:(kc + 1) * 128, :])
        wih.append(t)
    ps_g = ps.tile([128, 4], f32, tag="mm", bufs=3)
    for g in range(4):
        for kc in range(KX):
            nc.tensor.matmul(ps_g[:, g:g + 1], wih[kc][:, g * 128:(g + 1) * 128],
                             xk[:, kc:kc + 1], start=(kc == 0), stop=(kc == KX - 1))
    sig_i = smp.tile([128, 1], f32, tag="sA")
    nc.scalar.activation(sig_i[:], ps_g[:, 0:1], AF.Sigmoid, bias=hb[:, 0:1])
    sig_f = smp.tile([128, 1], f32, tag="sB")
    nc.scalar.activation(sig_f[:], ps_g[:, 1:2], AF.Sigmoid, bias=hb[:, 1:2])
    tan_g = smp.tile([128, 1], f32, tag="sC")
    nc.scalar.activation(tan_g[:], ps_g[:, 2:3], AF.Tanh, bias=hb[:, 2:3])
    sig_o = smp.tile([128, 1], f32, tag="sD")
    nc.scalar.activation(sig_o[:], ps_g[:, 3:4], AF.Sigmoid, bias=hb[:, 3:4])
    t1 = smp.tile([128, 1], f32, tag="sE")
    nc.vector.tensor_mul(t1[:], sig_f[:], c0[:])
    t2 = smp.tile([128, 1], f32, tag="sF")
    nc.vector.tensor_mul(t2[:], sig_i[:], tan_g[:])
    c_sb = smp.tile([128, 1], f32, tag="sG")
    nc.vector.tensor_add(c_sb[:], t1[:], t2[:])
    tan_c = smp.tile([128, 1], f32, tag="sH")
    nc.scalar.activation(tan_c[:], c_sb[:], AF.Tanh)
    ht_sb = smp.tile([128, 1], f32, tag="sI")
    nc.vector.tensor_mul(ht_sb[:], sig_o[:], tan_c[:])
    dma(O["hout"], ht_sb[:])
    dma(O["cout"], c_sb[:])

    # AllGather ht -> full ht in K-layout [128, 8]
    ag_ht_i = dram.tile([128, 1], f32, tag="aghti")
    ag_ht_o = dram.tile([H, 1], f32, tag="aghto")
    dma(ag_ht_i[:], ht_sb[:])
    nc.gpsimd.collective_compute("AllGather", ALU.bypass,
                                 replica_groups=[list(range(NCORES))],
                                 ins=[ag_ht_i[:]], outs=[ag_ht_o[:]])
    ht8 = smp.tile([128, KH], f32, tag="ht8")
    dma(ht8[:], ag_ht_o[:].rearrange("(j p) x -> p (j x)", p=128))

    # ================= Stage B: u_k = Wa_h[chunk] @ ht + b_attn =========
    wath = []
    for kc in range(KH):
        t = wathp.tile([128, 128], f32, tag=f"wath{kc}")
        dma(t[:], I["wathT"][kc * 128:(kc + 1) * 128, :])
        wath.append(t)
    ps_u = ps.tile([128, 1], f32, tag="mm", bufs=3)
    for kc in range(KH):
        nc.tensor.matmul(ps_u[:], wath[kc][:], ht8[:, kc:kc + 1],
                         start=(kc == 0), stop=(kc == KH - 1))
    u_sb = smp.tile([128, 1], f32, tag="sJ")
    nc.scalar.activation(u_sb[:], ps_u[:], AF.Identity, bias=battn[:])
    ag_u_i = dram.tile([128, 1], f32, tag="agui")
    ag_u_o = dram.tile([H, 1], f32, tag="aguo")
    dma(ag_u_i[:], u_sb[:])
    nc.gpsimd.collective_compute("AllGather", ALU.bypass,
                                 replica_groups=[list(range(NCORES))],
                                 ins=[ag_u_i[:]], outs=[ag_u_o[:]])
    u8 = smp.tile([128, KH], f32, tag="u8")
    dma(u8[:], ag_u_o[:].rearrange("(j p) x -> p (j x)", p=128))

    # ====== Stage C: Z.T = Wa_e @ enc_k.T ; scores_k = tanh(Z.T+u) @ v ==
    wae, enct = [], []
    for ec in range(KH):
        t = waep.tile([128, H], f32, tag=f"wae{ec}")
        dma(t[:], I["waeT"][ec * 128:(ec + 1) * 128, :])
        wae.append(t)
        t2_ = enctp.tile([128, SC], f32, tag=f"enct{ec}")
        dma(t2_[:], I["encT"][ec * 128:(ec + 1) * 128, :])
        enct.append(t2_)
    tz = []
    for hc in range(KH):
        ps_zt = ps.tile([128, SC], f32, tag="zt", bufs=2)
        for ec in range(KH):
            nc.tensor.matmul(ps_zt[:], wae[ec][:, hc * 128:(hc + 1) * 128],
                             enct[ec][:], start=(ec == 0), stop=(ec == KH - 1))
        t = tzp.tile([128, SC], f32, tag=f"tz{hc}")
        nc.scalar.activation(t[:], ps_zt[:], AF.Tanh, bias=u8[:, hc:hc + 1])
        tz.append(t)
    ps_s = ps.tile([128, 2], f32, tag="mm", bufs=3)
    for j in range(SC // 128):
        for hc in range(KH):
            nc.tensor.matmul(ps_s[:, j:j + 1], tz[hc][:, j * 128:(j + 1) * 128],
                             vk[:, hc:hc + 1], start=(hc == 0), stop=(hc == KH - 1))
    exp_sc = smp.tile([128, 2], f32, tag="expsc")
    zrow = smp.tile([128, 1], f32, tag="zrow")
    nc.scalar.activation(exp_sc[:], ps_s[:], AF.Exp, accum_out=zrow[:])
    ecz = smp.tile([1, 1032], f32, tag="ecz")
    nc.vector.memset(ecz[:], 0.0)
    ps_z1 = ps.tile([1, 1], f32, tag="ec", bufs=3)
    nc.tensor.matmul(ps_z1[:], zrow[:], ones[:], start=True, stop=True)
    nc.scalar.activation(ecz[:, 1024:1025], ps_z1[:], AF.Copy)

    # unnormalized context partial: ectx = exp_sc.T @ enc_k   [1, 1024]
    encn = []
    for j in range(SC // 128):
        t = encnp.tile([128, E], f32, tag=f"encn{j}")
        dma(t[:], I["encN"][j * 128:(j + 1) * 128, :])
        encn.append(t)
    ps_e = []
    for nh in range(2):
        p = ps.tile([1, 512], f32, tag="ec", bufs=3)
        for j in range(SC // 128):
            nc.tensor.matmul(p[:], exp_sc[:, j:j + 1],
                             encn[j][:, nh * 512:(nh + 1) * 512],
                             start=(j == 0), stop=(j == SC // 128 - 1))
        ps_e.append(p)
    nc.scalar.activation(ecz[:, 0:512], ps_e[0][:], AF.Copy)
    nc.scalar.activation(ecz[:, 512:1024], ps_e[1][:], AF.Copy)
    ar_i = dram.tile([1, 1032], f32, tag="ari")
    ar_o = dram.tile([1, 1032], f32, tag="aro")
    dma(ar_i[:], ecz[:])
    nc.gpsimd.collective_compute("AllReduce", ALU.add,
                                 replica_groups=[list(range(NCORES))],
                                 ins=[ar_i[:]], outs=[ar_o[:]])
    ectx8 = smp.tile([128, KH], f32, tag="ectx8")
    dma(ectx8[:], ar_o[0:1, 0:1024].rearrange("x (j p) -> p (x j)", p=128))
    zg = smp.tile([1, 1], f32, tag="zg")
    dma(zg[:], ar_o[0:1, 1024:1025])
    rz = smp.tile([1, 1], f32, tag="rz")
    nc.vector.reciprocal(rz[:], zg[:])
    rz_d = dram.tile([1, 1], f32, tag="rzd")
    dma(rz_d[:], rz[:])
    rzb = smp.tile([128, 1], f32, tag="rzb")
    dma(rzb[:], rz_d[:].to_broadcast((128, 1)))
    ctx8 = smp.tile([128, KH], f32, tag="ctx8")
    nc.vector.tensor_scalar_mul(ctx8[:], ectx8[:], rzb[:])
    w_sb = smp.tile([128, 2], f32, tag="wsb")
    nc.vector.tensor_scalar_mul(w_sb[:], exp_sc[:], rzb[:])
    dma(O["wout"], w_sb[:])

    # ========== Stage E: ht_tilda_k = tanh(W_ah[chunk] @ [ctx; ht]) =====
    wah = []
    for kc in range(KX):
        t = wahp.tile([128, 128], f32, tag=f"wah{kc % 8}")
        dma(t[:], I["wahT"][kc * 128:(kc + 1) * 128, :])
        wah.append(t)
    ps_h = ps.tile([128, 1], f32, tag="mm", bufs=3)
    for kc in range(KX):
        rhs = ctx8[:, kc:kc + 1] if kc < KH else ht8[:, kc - KH:kc - KH + 1]
        nc.tensor.matmul(ps_h[:], wah[kc][:], rhs,
                         start=(kc == 0), stop=(kc == KX - 1))
    htt = smp.tile([128, 1], f32, tag="sK")
    nc.scalar.activation(htt[:], ps_h[:], AF.Tanh, bias=bah[:])
    dma(O["httout"], htt[:])
    ag_t_i = dram.tile([128, 1], f32, tag="agti")
    ag_t_o = dram.tile([H, 1], f32, tag="agto")
    dma(ag_t_i[:], htt[:])
    nc.gpsimd.collective_compute("AllGather", ALU.bypass,
                                 replica_groups=[list(range(NCORES))],
                                 ins=[ag_t_i[:]], outs=[ag_t_o[:]])
    htt8 = smp.tile([128, KH], f32, tag="htt8")
    dma(htt8[:], ag_t_o[:].rearrange("(j p) x -> p (j x)", p=128))

    # ====== Stage F: logits_k = W_out[shard] @ ht_tilda ; log_softmax ===
    ps_lg = ps.tile([128, 32], f32, tag="mm", bufs=3)
    for g in range(8):
        wo = wop.tile([128, 4096], f32, tag="wo")
        dma(wo[:], I["woutP"][g, :, :])
        for sub in range(4):
            mc = g * 4 + sub
            for ec in range(KH):
                nc.tensor.matmul(ps_lg[:, mc:mc + 1],
                                 wo[:, sub * 1024 + ec * 128: sub * 1024 + (ec + 1) * 128],
                                 htt8[:, ec:ec + 1],
                                 start=(ec == 0), stop=(ec == KH - 1))
    lg_sb = smp.tile([128, 32], f32, tag="lgsb")
    nc.vector.tensor_add(lg_sb[:], ps_lg[:], bout[:])
    exp2 = smp.tile([128, 32], f32, tag="exp2")
    z2row = smp.tile([128, 1], f32, tag="z2row")
    nc.scalar.activation(exp2[:], lg_sb[:], AF.Exp, accum_out=z2row[:])
    ps_z2 = ps.tile([1, 1], f32, tag="ec", bufs=3)
    nc.tensor.matmul(ps_z2[:], z2row[:], ones[:], start=True, stop=True)
    z2_sb = smp.tile([1, 1], f32, tag="z2sb")
    nc.scalar.activation(z2_sb[:], ps_z2[:], AF.Copy)
    ag_z_i = dram.tile([1, 1], f32, tag="agzi")
    ag_z_o = dram.tile([NCORES, 1], f32, tag="agzo")
    dma(ag_z_i[:], z2_sb[:])
    nc.gpsimd.collective_compute("AllGather", ALU.bypass,
                                 replica_groups=[list(range(NCORES))],
                                 ins=[ag_z_i[:]], outs=[ag_z_o[:]])
    z2all = smp.tile([1, NCORES], f32, tag="z2all")
    dma(z2all[:], ag_z_o[:].rearrange("(x j) y -> x (j y)", x=1))
    z2g = smp.tile([1, 1], f32, tag="z2g")
    nc.vector.reduce_sum(z2g[:], z2all[:], axis=mybir.AxisListType.X)
    lnz = smp.tile([1, 1], f32, tag="lnz")
    nc.scalar.activation(lnz[:], z2g[:], AF.Ln)
    nlz = smp.tile([1, 1], f32, tag="nlz")
    nc.scalar.mul(nlz[:], lnz[:], -1.0)
    nl_d = dram.tile([1, 1], f32, tag="nld")
    dma(nl_d[:], nlz[:])
    nlb = smp.tile([128, 1], f32, tag="nlb")
    dma(nlb[:], nl_d[:].to_broadcast((128, 1)))
    outp = smp.tile([128, 32], f32, tag="outp")
    nc.scalar.activation(outp[:], lg_sb[:], AF.Identity, bias=nlb[:])
    dma(O["lpout"], outp[:])


pools_ones = [None]


def build_module(n_iters=1, wo_bufs=5):
    """Build + compile the SPMD module. Returns (nc, input names)."""
    nc = bacc.Bacc("TRN2", target_bir_lowering=False, debug=False,
                   enable_asserts=False, num_devices=NCORES)
    I = {
        "xk":    nc.dram_tensor("xk", [128, KX], f32, kind="ExternalInput").ap(),
        "hbias": nc.dram_tensor("hbias", [128, 4], f32, kind="ExternalInput").ap(),
        "c0k":   nc.dram_tensor("c0k", [128, 1], f32, kind="ExternalInput").ap(),
        "wihT":  nc.dram_tensor("wihT", [E + H, 4 * 128], f32, kind="ExternalInput").ap(),
        "wathT": nc.dram_tensor("wathT", [H, 128], f32, kind="ExternalInput").ap(),
        "battn": nc.dram_tensor("battn", [128, 1], f32, kind="ExternalInput").ap(),
        "waeT":  nc.dram_tensor("waeT", [E, H], f32, kind="ExternalInput").ap(),
        "encT":  nc.dram_tensor("encT", [E, SC], f32, kind="ExternalInput").ap(),
        "encN":  nc.dram_tensor("encN", [SC, E], f32, kind="ExternalInput").ap(),
        "vk":    nc.dram_tensor("vk", [128, KH], f32, kind="ExternalInput").ap(),
        "wahT":  nc.dram_tensor("wahT", [E + H, 128], f32, kind="ExternalInput").ap(),
        "bah":   nc.dram_tensor("bah", [128, 1], f32, kind="ExternalInput").ap(),
        "woutP": nc.dram_tensor("woutP", [8, 128, 4096], f32, kind="ExternalInput").ap(),
        "bout":  nc.dram_tensor("bout", [128, 32], f32, kind="ExternalInput").ap(),
    }
    O = {
        "hout":   nc.dram_tensor("hout", [128, 1], f32, kind="ExternalOutput").ap(),
        "cout":   nc.dram_tensor("cout", [128, 1], f32, kind="ExternalOutput").ap(),
        "httout": nc.dram_tensor("httout", [128, 1], f32, kind="ExternalOutput").ap(),
        "wout":   nc.dram_tensor("wout", [128, 2], f32, kind="ExternalOutput").ap(),
        "lpout":  nc.dram_tensor("lpout", [128, 32], f32, kind="ExternalOutput").ap(),
    }
    with tile.TileContext(nc) as tc:
        with tc.tile_pool(name="const", bufs=1) as const, \
             tc.tile_pool(name="wih", bufs=2) as wihp, \
             tc.tile_pool(name="wath", bufs=1) as wathp, \
             tc.tile_pool(name="wae", bufs=1) as waep, \
             tc.tile_pool(name="enct", bufs=1) as enctp, \
             tc.tile_pool(name="encn", bufs=1) as encnp, \
             tc.tile_pool(name="tz", bufs=1) as tzp, \
             tc.tile_pool(name="wah", bufs=2) as wahp, \
             tc.tile_pool(name="wo", bufs=wo_bufs) as wop, \
             tc.tile_pool(name="sm", bufs=2) as smp, \
             tc.tile_pool(name="dram", bufs=2, space="DRAM") as dram, \
             tc.tile_pool(name="psum", bufs=1, space="PSUM") as ps:
            ones = const.tile([128, 1], f32, tag="ones")
            nc.vector.memset(ones[:], 1.0)
            pools_ones[0] = ones
            pools = (const, wihp, wathp, waep, enctp, encnp, tzp, wahp, wop,
                     smp, dram, ps)
            for _ in range(n_iters):
                _emit_iter(nc, tc, I, O, pools)
    nc.compile()
    nc.m = get_hw_module(nc.m)
    return nc


# ---------------------------------------------------------------- host side
def shard_inputs(encoder_out, word_input, last_context, h0, c0,
                 emb, W_ih, W_hh, b_ih, b_hh,
                 W_attn, b_attn, v, W_ah, b_ah, W_out, b_out):
    """Host-side preprocessing: embedding lookup, weight transposes/packing,
    per-core sharding. Returns list of 8 in_map dicts."""
    f = np.float32
    enc = np.asarray(encoder_out, f)
    word = int(np.asarray(word_input).reshape(-1)[0])
    embedded = np.asarray(emb, f)[word]                     # [E]
    x = np.concatenate([np.asarray(last_context, f)[0], embedded])  # [H+E]
    xk = np.ascontiguousarray(x.reshape(KX, 128).T)          # [128, KX]

    h0v = np.asarray(h0, f)[0, 0]
    c0v = np.asarray(c0, f)[0, 0]
    hbias = np.asarray(b_ih, f) + np.asarray(b_hh, f)
    if h0v.any():
        hbias = hbias + np.asarray(W_hh, f) @ h0v

    W_ih = np.asarray(W_ih, f)
    W_attn = np.asarray(W_attn, f)
    Wa_h, Wa_e = W_attn[:, :H], W_attn[:, H:]
    waeT = np.ascontiguousarray(Wa_e.T)                      # [E, H]
    W_ah = np.asarray(W_ah, f)
    W_out = np.asarray(W_out, f)
    b_out = np.asarray(b_out, f)
    v = np.asarray(v, f)
    vk = np.ascontiguousarray(v.reshape(KH, 128).T)          # [128, KH]

    in_maps = []
    for k in range(NCORES):
        hs = slice(k * HC, (k + 1) * HC)
        rows = np.concatenate([np.arange(g * H + k * HC, g * H + (k + 1) * HC)
                               for g in range(4)])
        wihT = np.ascontiguousarray(W_ih[rows, :].T)         # [E+H, 512]
        hb_k = np.ascontiguousarray(hbias[rows].reshape(4, HC).T)  # [128,4]
        c0_k = np.ascontiguousarray(c0v[hs].reshape(HC, 1))
        wathT = np.ascontiguousarray(Wa_h[hs, :].T)          # [H, 128]
        battn_k = np.ascontiguousarray(np.asarray(b_attn, f)[hs].reshape(HC, 1))
        encT_k = np.ascontiguousarray(enc[k * SC:(k + 1) * SC, :].T)  # [E, SC]
        encN_k = np.ascontiguousarray(enc[k * SC:(k + 1) * SC, :])    # [SC, E]
        wahT = np.ascontiguousarray(W_ah[hs, :].T)           # [E+H, 128]
        bah_k = np.ascontiguousarray(np.asarray(b_ah, f)[hs].reshape(HC, 1))
        # vocab shard, padded, packed as [g, p(e-local), (sub, ec, q)]
        wo_pad = np.zeros((VP, H), f)
        wo_pad[:VC] = W_out[k * VC:(k + 1) * VC, :]
        woT = wo_pad.T                                        # [H(e), VP(v)]
        # [ec, p, mc, q] -> group: [g, sub, p, ec, q] -> [g, p, sub, ec, q]
        w4 = woT.reshape(KH, 128, 32, 128)                    # [ec, p, mc, q]
        w4 = w4.transpose(2, 1, 0, 3)                         # [mc, p, ec, q]
        w4 = w4.reshape(8, 4, 128, KH, 128).transpose(0, 2, 1, 3, 4)
        woutP = np.ascontiguousarray(w4.reshape(8, 128, 4096))
        bo_pad = np.full(VP, NEG_BIG, f)
        bo_pad[:VC] = b_out[k * VC:(k + 1) * VC]
        bout_k = np.ascontiguousarray(bo_pad.reshape(32, 128).T)  # [128, 32]
        in_maps.append({
            "xk": xk, "hbias": hb_k, "c0k": c0_k, "wihT": wihT,
            "wathT": wathT, "battn": battn_k, "waeT": waeT,
            "encT": encT_k, "encN": encN_k, "vk": vk,
            "wahT": wahT, "bah": bah_k, "woutP": woutP, "bout": bout_k,
        })
    return in_maps


def assemble_outputs(results):
    """results: list (per core) of dicts of output arrays -> reference pytree."""
    f = np.float32
    ht = np.concatenate([results[k]["hout"].reshape(-1) for k in range(NCORES)])
    c = np.concatenate([results[k]["cout"].reshape(-1) for k in range(NCORES)])
    htt = np.concatenate([results[k]["httout"].reshape(-1) for k in range(NCORES)])
    w = np.concatenate(
        [results[k]["wout"].T.reshape(-1) for k in range(NCORES)])
    out = np.concatenate(
        [results[k]["lpout"].T.reshape(-1)[:VC] for k in range(NCORES)])
    return (out[None, :].astype(f),
            (ht[None, None, :].astype(f), c[None, None, :].astype(f)),
            htt[None, :].astype(f), w[None, :].astype(f))


_cached_nc = None


def kernel(**inputs):
    global _cached_nc
    if _cached_nc is None:
        _cached_nc = build_module(n_iters=1)
    in_maps = shard_inputs(**inputs)
    res = bass_utils.run_bass_kernel_spmd(
        _cached_nc, in_maps, core_ids=list(range(NCORES)))
    return assemble_outputs(res.results)


if __name__ == "__main__":
    import reference
    inputs = {k: np.asarray(val) for k, val in reference.setup_inputs().items()}
    expected = reference.reference(**inputs)
    actual = kernel(**inputs)
    import jax
    for (ep, e), (ap_, a) in zip(
            jax.tree_util.tree_leaves_with_path(expected),
            jax.tree_util.tree_leaves_with_path(actual)):
        e = np.asarray(e); a = np.asarray(a)
        rel = np.abs(a - e).max() / (np.abs(e).max() + 1e-12)
        print(f"{jax.tree_util.keystr(ep)}: rel={rel:.3e}")


# revision 11
# speedup vs baseline: 6.3046x; 6.3046x over previous
"""Trainium2 Bass kernel for a single-step attention decoder (DecoderRNN).

Computation (batch=1, single decode step):
  embedded = emb[word]                                  [E]
  x  = concat(last_context, embedded)                   [H+E]
  gates = W_ih @ x + b_ih + W_hh @ h0 + b_hh            [4H] (i,f,g,o)
  c  = sig(f)*c0 + sig(i)*tanh(g);  ht = sig(o)*tanh(c) [H]
  scores = tanh(enc @ Wa_e.T + (Wa_h @ ht + b_attn)) @ v  [S]
  w  = softmax(scores);  context = w @ enc              [E]
  ht_tilda = tanh(W_ah @ concat(context, ht) + b_ah)    [H]
  out = log_softmax(W_out @ ht_tilda + b_out)           [V]

Sharding across 8 NeuronCores, built to minimize the serial chain (three tiny
collectives total, no strided gathers):
  - LSTM rows hidden-sharded: core k produces ht_k, c_k [128].
  - Wa_h and W_ah[:,H:] are COLUMN-sharded: each core turns its own ht_k into
    full-length partial vectors u^k = Wa_h[:,k]@ht_k and w2^k = W_ah[:,H+k]@ht_k;
    AllReduce#1 sums both (payload [128,16], K-layout, contiguous).
  - encoder_out sequence-sharded for scores; exp(scores) partials give an
    unnormalized context partial [128,8] + local softmax normalizer;
    AllReduce#2 sums them (payload [128,9]).
  - ht_tilda computed FULLY on every core (replicated W_ah[:,:H] @ context
    + the AllReduced w2) -> feeds the vocab-sharded W_out matvec directly.
  - log_softmax normalizer via AllGather#3 of per-core sum(exp(logits)) (8 B).
Weight matvecs run on the PE with host-pre-transposed (optionally bf16)
weights; no max-subtraction needed (logits are O(1) by construction).
"""
import numpy as np

import concourse.bass as bass
import concourse.mybir as mybir
import concourse.tile as tile
from concourse import bacc, bass_utils
from concourse.bass_interp import get_hw_module

NCORES = 8
V, E, H, S = 32000, 1024, 1024, 2048
HC = H // NCORES      # 128  hidden chunk per core
SC = S // NCORES      # 256  sequence chunk per core
VC = V // NCORES      # 4000 vocab rows per core
VP = 4096             # padded vocab shard (32 tiles of 128)
KX = (E + H) // 128   # 16   contraction chunks for x
KH = H // 128         # 8    contraction chunks for ht-sized vectors
NEG_BIG = -1.0e5      # pad-row bias: exp() underflows to exactly 0

f32 = mybir.dt.float32
bf16 = mybir.dt.bfloat16
AF = mybir.ActivationFunctionType
ALU = mybir.AluOpType

# dtype config for the heavy weight streams (host casts to match)
WIH_DT = f32    # W_ih (gates)
WO_DT = bf16    # W_out (vocab projection)
WAHC_DT = f32   # W_ah[:, :H] (context part, replicated)
WSM_DT = f32    # Wa_h / W_ah[:,H:] column shards (small)
ATT_DT = f32    # Wa_e / enc (scores + context path stays f32 for w precision)

_G = {}  # build-scoped globals (ones tiles)


def _np_dt(dt):
    return np.float32 if dt == f32 else np.dtype("bfloat16")


# ---------------------------------------------------------------- device code
def _emit_iter(nc, tc, I, O, pools, stop_after=None):
    (wihp, wattp, waep, enctp, encnp, tzp, wahcp, wop, smp, dram, ps) = pools

    def dma(dst, src):
        nc.sync.dma_start(dst, src)

    ones = _G["ones"]        # [128,1] f32 1.0
    one1 = _G["one1"]        # [1,1] WIH_DT 1.0

    # ---- small constants ----
    xk = smp.tile([128, KX], WIH_DT, tag="xk")
    dma(xk[:], I["xk"])
    hbr = smp.tile([1, 512], WIH_DT, tag="hbr")
    dma(hbr[:], I["hbiasr"])
    c0 = smp.tile([128, 1], f32, tag="c0")
    dma(c0[:], I["c0k"])
    ub8 = smp.tile([128, KH], f32, tag="ub8")   # b_attn in K-layout
    dma(ub8[:], I["battn8"])
    bah8 = smp.tile([128, KH], f32, tag="bah8")
    dma(bah8[:], I["bah8"])
    vk = smp.tile([128, KH], f32, tag="vk")
    dma(vk[:], I["vk"])
    bout = smp.tile([128, 32], f32, tag="bout")
    dma(bout[:], I["bout"])

    # ================= Stage A: LSTM gates -> ht_k, c_k =================
    # gate columns: 0=i, 1=f, 2=o, 3=g (host reordered)
    wih = []
    for kc in range(KX):
        t = wihp.tile([128, 512], WIH_DT, tag=f"wih{kc % 8}", bufs=2)
        dma(t[:], I["wihT"][kc * 128:(kc + 1) * 128, :])
        wih.append(t)
    ps_g = ps.tile([128, 4], f32, tag="mm", bufs=3)
    for g in range(4):
        nc.tensor.matmul(ps_g[:, g:g + 1], hbr[:, g * 128:(g + 1) * 128],
                         one1[:], start=True, stop=False)
        for kc in range(KX):
            nc.tensor.matmul(ps_g[:, g:g + 1], wih[kc][:, g * 128:(g + 1) * 128],
                             xk[:, kc:kc + 1], start=False, stop=(kc == KX - 1))
    # sigmoid(x) = 0.5*tanh(0.5x)+0.5 for i,f,o; tanh for g — one table set.
    th3 = smp.tile([128, 3], f32, tag="th3")
    nc.scalar.activation(th3[:], ps_g[:, 0:3], AF.Tanh, scale=0.5)
    tan_g = smp.tile([128, 1], f32, tag="tang")
    nc.scalar.activation(tan_g[:], ps_g[:, 3:4], AF.Tanh)
    sg3 = smp.tile([128, 3], f32, tag="sg3")
    nc.vector.tensor_scalar(sg3[:], th3[:], 1.0, 0.5, ALU.add, ALU.mult)
    t1 = smp.tile([128, 1], f32, tag="sE")
    nc.vector.tensor_mul(t1[:], sg3[:, 1:2], c0[:])      # sig_f * c0
    t2 = smp.tile([128, 1], f32, tag="sF")
    nc.vector.tensor_mul(t2[:], sg3[:, 0:1], tan_g[:])   # sig_i * tanh(g)
    c_sb = smp.tile([128, 1], f32, tag="sG")
    nc.vector.tensor_add(c_sb[:], t1[:], t2[:])
    tan_c = smp.tile([128, 1], f32, tag="sH")
    nc.scalar.activation(tan_c[:], c_sb[:], AF.Tanh)
    ht_sb = smp.tile([128, 1], f32, tag="sI")
    nc.vector.tensor_mul(ht_sb[:], sg3[:, 2:3], tan_c[:])  # sig_o * tanh(c)
    dma(O["hout"], ht_sb[:])
    dma(O["cout"], c_sb[:])
    if stop_after == "A":
        return

    # ===== Stage B: partial u^k, w2^k (column-sharded matvecs) + AR1 =====
    htb = smp.tile([128, 1], WSM_DT, tag="htb")
    nc.vector.tensor_copy(htb[:], ht_sb[:])
    watt = wattp.tile([128, 2048], WSM_DT, tag="watt")   # [Wa_h col | W_ah col]
    dma(watt[:], I["wattT"])
    ps_uw = ps.tile([128, 16], f32, tag="mm", bufs=3)
    for mt in range(16):
        nc.tensor.matmul(ps_uw[:, mt:mt + 1], watt[:, mt * 128:(mt + 1) * 128],
                         htb[:], start=True, stop=True)
    uw_sb = smp.tile([128, 16], f32, tag="uwsb")
    nc.scalar.activation(uw_sb[:], ps_uw[:], AF.Copy)
    ar1_i = dram.tile([128, 16], f32, tag="ar1i")
    ar1_o = dram.tile([128, 16], f32, tag="ar1o")
    dma(ar1_i[:], uw_sb[:])
    nc.gpsimd.collective_compute("AllReduce", ALU.add,
                                 replica_groups=[list(range(NCORES))],
                                 ins=[ar1_i[:]], outs=[ar1_o[:]])
    uw8 = smp.tile([128, 16], f32, tag="uw8")
    dma(uw8[:], ar1_o[:])
    # u8 = u + b_attn (K-layout [128,8])
    u8 = smp.tile([128, KH], f32, tag="u8")
    nc.vector.tensor_add(u8[:], uw8[:, 0:KH], ub8[:])
    if stop_after == "B":
        dma(O["httout"], u8[:])
        return

    # ====== Stage C: Z.T = Wa_e @ enc_k.T ; scores_k = tanh(Z.T+u) @ v ==
    wae, enct = [], []
    for ec in range(KH):
        t = waep.tile([128, H], ATT_DT, tag=f"wae{ec}")
        dma(t[:], I["waeT"][ec * 128:(ec + 1) * 128, :])
        wae.append(t)
        t2_ = enctp.tile([128, SC], ATT_DT, tag=f"enct{ec}")
        dma(t2_[:], I["encT"][ec * 128:(ec + 1) * 128, :])
        enct.append(t2_)
    tz = []
    for hc in range(KH):
        ps_zt = ps.tile([128, SC], f32, tag="zt", bufs=2)
        for ec in range(KH):
            nc.tensor.matmul(ps_zt[:], wae[ec][:, hc * 128:(hc + 1) * 128],
                             enct[ec][:], start=(ec == 0), stop=(ec == KH - 1))
        t = tzp.tile([128, SC], f32, tag=f"tz{hc}")
        nc.scalar.activation(t[:], ps_zt[:], AF.Tanh, bias=u8[:, hc:hc + 1])
        tz.append(t)
    ps_s = ps.tile([128, 2], f32, tag="mm", bufs=3)
    for j in range(SC // 128):
        for hc in range(KH):
            nc.tensor.matmul(ps_s[:, j:j + 1], tz[hc][:, j * 128:(j + 1) * 128],
                             vk[:, hc:hc + 1], start=(hc == 0), stop=(hc == KH - 1))
    exp_sc = smp.tile([128, 2], f32, tag="expsc")
    zrow = smp.tile([128, 1], f32, tag="zrow")
    nc.scalar.activation(exp_sc[:], ps_s[:], AF.Exp, accum_out=zrow[:])

    # ===== Stage D: unnormalized context partial [128,8] + z ; AR2 =====
    encn = []
    for j in range(SC // 128):
        t = encnp.tile([128, E], ATT_DT, tag=f"encn{j}")
        dma(t[:], I["encN"][j * 128:(j + 1) * 128, :])
        encn.append(t)
    ps_ec = ps.tile([128, KH], f32, tag="mm", bufs=3)
    for mt in range(KH):
        for j in range(SC // 128):
            nc.tensor.matmul(ps_ec[:, mt:mt + 1],
                             encn[j][:, mt * 128:(mt + 1) * 128],
                             exp_sc[:, j:j + 1],
                             start=(j == 0), stop=(j == SC // 128 - 1))
    ps_z1 = ps.tile([1, 1], f32, tag="ec", bufs=2)
    nc.tensor.matmul(ps_z1[:], zrow[:], ones[:], start=True, stop=True)
    ecz = smp.tile([128, 9], f32, tag="ecz")
    nc.vector.memset(ecz[:, 8:9], 0.0)
    nc.scalar.activation(ecz[:, 0:8], ps_ec[:], AF.Copy)
    nc.scalar.activation(ecz[0:1, 8:9], ps_z1[:], AF.Copy)
    ar2_i = dram.tile([128, 9], f32, tag="ar2i")
    ar2_o = dram.tile([128, 9], f32, tag="ar2o")
    dma(ar2_i[:], ecz[:])
    nc.gpsimd.collective_compute("AllReduce", ALU.add,
                                 replica_groups=[list(range(NCORES))],
                                 ins=[ar2_i[:]], outs=[ar2_o[:]])
    ectx8 = smp.tile([128, KH], f32, tag="ectx8")
    dma(ectx8[:], ar2_o[:, 0:8])
    zb = smp.tile([128, 1], f32, tag="zb")
    dma(zb[:], ar2_o[0:1, 8:9].to_broadcast((128, 1)))
    rzb = smp.tile([128, 1], f32, tag="rzb")
    nc.vector.reciprocal(rzb[:], zb[:])
    ctx8 = smp.tile([128, KH], WAHC_DT, tag="ctx8")
    nc.vector.tensor_scalar_mul(ctx8[:], ectx8[:], rzb[:])
    w_sb = smp.tile([128, 2], f32, tag="wsb")
    nc.vector.tensor_scalar_mul(w_sb[:], exp_sc[:], rzb[:])
    dma(O["wout"], w_sb[:])
    if stop_after == "C":
        dma(O["httout"], ectx8[:])
        return

    # ===== Stage E: full ht_tilda = tanh(Wah_c@ctx + w2 + b_ah) [128,8] ==
    wahc = []
    for kc in range(KH):
        t = wahcp.tile([128, H], WAHC_DT, tag=f"wahc{kc}")
        dma(t[:], I["wahcT"][kc * 128:(kc + 1) * 128, :])
        wahc.append(t)
    ps_ht = ps.tile([128, KH], f32, tag="mm", bufs=3)
    for mt in range(KH):
        for kc in range(KH):
            nc.tensor.matmul(ps_ht[:, mt:mt + 1],
                             wahc[kc][:, mt * 128:(mt + 1) * 128],
                             ctx8[:, kc:kc + 1],
                             start=(kc == 0), stop=(kc == KH - 1))
    w2b = smp.tile([128, KH], f32, tag="w2b")
    nc.vector.tensor_add(w2b[:], uw8[:, KH:16], bah8[:])
    htsum = smp.tile([128, KH], f32, tag="htsum")
    nc.vector.tensor_add(htsum[:], ps_ht[:], w2b[:])
    htt8 = smp.tile([128, KH], WO_DT, tag="htt8")
    nc.scalar.activation(htt8[:], htsum[:], AF.Tanh)
    htt8f = smp.tile([128, KH], f32, tag="htt8f")
    nc.scalar.activation(htt8f[:], htsum[:], AF.Tanh)
    dma(O["httout"], htt8f[:])
    if stop_after == "E":
        return

    # ====== Stage F: logits_k = W_out[shard] @ ht_tilda ; log_softmax ===
    ps_lg = ps.tile([128, 32], f32, tag="mm", bufs=3)
    for g in range(8):
        wo = wop.tile([128, 4096], WO_DT, tag="wo")
        dma(wo[:], I["woutP"][g, :, :])
        for sub in range(4):
            mc = g * 4 + sub
            for ec in range(KH):
                nc.tensor.matmul(ps_lg[:, mc:mc + 1],
                                 wo[:, sub * 1024 + ec * 128: sub * 1024 + (ec + 1) * 128],
                                 htt8[:, ec:ec + 1],
                                 start=(ec == 0), stop=(ec == KH - 1))
    lg_sb = smp.tile([128, 32], f32, tag="lgsb")
    nc.vector.tensor_add(lg_sb[:], ps_lg[:], bout[:])
    exp2 = smp.tile([128, 32], f32, tag="exp2")
    z2row = smp.tile([128, 1], f32, tag="z2row")
    nc.scalar.activation(exp2[:], lg_sb[:], AF.Exp, accum_out=z2row[:])
    ps_z2 = ps.tile([1, 1], f32, tag="ec", bufs=2)
    nc.tensor.matmul(ps_z2[:], z2row[:], ones[:], start=True, stop=True)
    z2_sb = smp.tile([1, 1], f32, tag="z2sb")
    nc.scalar.activation(z2_sb[:], ps_z2[:], AF.Copy)
    ag_z_i = dram.tile([1, 1], f32, tag="agzi")
    ag_z_o = dram.tile([NCORES, 1], f32, tag="agzo")
    dma(ag_z_i[:], z2_sb[:])
    nc.gpsimd.collective_compute("AllGather", ALU.bypass,
                                 replica_groups=[list(range(NCORES))],
                                 ins=[ag_z_i[:]], outs=[ag_z_o[:]])
    z2all = smp.tile([128, NCORES], f32, tag="z2all")
    dma(z2all[:], ag_z_o[:].rearrange("(x j) y -> x (j y)", x=1).to_broadcast((128, NCORES)))
    z2g = smp.tile([128, 1], f32, tag="z2g")
    nc.vector.reduce_sum(z2g[:], z2all[:], axis=mybir.AxisListType.X)
    lnz = smp.tile([128, 1], f32, tag="lnz")
    nc.scalar.activation(lnz[:], z2g[:], AF.Ln)
    outp = smp.tile([128, 32], f32, tag="outp")
    nc.vector.tensor_scalar(outp[:], lg_sb[:], lnz[:], None, ALU.subtract)
    dma(O["lpout"], outp[:])


def build_module(n_iters=1, wo_bufs=7, stop_after=None):
    """Build + compile the SPMD module."""
    nc = bacc.Bacc("TRN2", target_bir_lowering=False, debug=False,
                   enable_asserts=False, num_devices=NCORES)
    I = {
        "xk":     nc.dram_tensor("xk", [128, KX], WIH_DT, kind="ExternalInput").ap(),
        "hbiasr": nc.dram_tensor("hbiasr", [1, 512], WIH_DT, kind="ExternalInput").ap(),
        "c0k":    nc.dram_tensor("c0k", [128, 1], f32, kind="ExternalInput").ap(),
        "wihT":   nc.dram_tensor("wihT", [E + H, 4 * 128], WIH_DT, kind="ExternalInput").ap(),
        "wattT":  nc.dram_tensor("wattT", [128, 2048], WSM_DT, kind="ExternalInput").ap(),
        "battn8": nc.dram_tensor("battn8", [128, KH], f32, kind="ExternalInput").ap(),
        "waeT":   nc.dram_tensor("waeT", [E, H], ATT_DT, kind="ExternalInput").ap(),
        "encT":   nc.dram_tensor("encT", [E, SC], ATT_DT, kind="ExternalInput").ap(),
        "encN":   nc.dram_tensor("encN", [SC, E], ATT_DT, kind="ExternalInput").ap(),
        "vk":     nc.dram_tensor("vk", [128, KH], f32, kind="ExternalInput").ap(),
        "wahcT":  nc.dram_tensor("wahcT", [E, H], WAHC_DT, kind="ExternalInput").ap(),
        "bah8":   nc.dram_tensor("bah8", [128, KH], f32, kind="ExternalInput").ap(),
        "woutP":  nc.dram_tensor("woutP", [8, 128, 4096], WO_DT, kind="ExternalInput").ap(),
        "bout":   nc.dram_tensor("bout", [128, 32], f32, kind="ExternalInput").ap(),
    }
    O = {
        "hout":   nc.dram_tensor("hout", [128, 1], f32, kind="ExternalOutput").ap(),
        "cout":   nc.dram_tensor("cout", [128, 1], f32, kind="ExternalOutput").ap(),
        "httout": nc.dram_tensor("httout", [128, KH], f32, kind="ExternalOutput").ap(),
        "wout":   nc.dram_tensor("wout", [128, 2], f32, kind="ExternalOutput").ap(),
        "lpout":  nc.dram_tensor("lpout", [128, 32], f32, kind="ExternalOutput").ap(),
    }
    with tile.TileContext(nc) as tc:
        with tc.tile_pool(name="wih", bufs=1) as wihp, \
             tc.tile_pool(name="watt", bufs=1) as wattp, \
             tc.tile_pool(name="wae", bufs=1) as waep, \
             tc.tile_pool(name="enct", bufs=1) as enctp, \
             tc.tile_pool(name="encn", bufs=1) as encnp, \
             tc.tile_pool(name="tz", bufs=1) as tzp, \
             tc.tile_pool(name="wahc", bufs=1) as wahcp, \
             tc.tile_pool(name="wo", bufs=wo_bufs) as wop, \
             tc.tile_pool(name="sm", bufs=2) as smp, \
             tc.tile_pool(name="const", bufs=1) as constp, \
             tc.tile_pool(name="dram", bufs=2, space="DRAM") as dram, \
             tc.tile_pool(name="psum", bufs=1, space="PSUM") as ps:
            ones = constp.tile([128, 1], f32, tag="ones")
            nc.vector.memset(ones[:], 1.0)
            one1 = constp.tile([1, 1], WIH_DT, tag="one1")
            nc.vector.memset(one1[:], 1.0)
            _G["ones"], _G["one1"] = ones, one1
            pools = (wihp, wattp, waep, enctp, encnp, tzp, wahcp, wop,
                     smp, dram, ps)
            for _ in range(n_iters):
                _emit_iter(nc, tc, I, O, pools, stop_after=stop_after)
    nc.compile()
    nc.m = get_hw_module(nc.m)
    return nc


# ---------------------------------------------------------------- host side
def shard_inputs(encoder_out, word_input, last_context, h0, c0,
                 emb, W_ih, W_hh, b_ih, b_hh,
                 W_attn, b_attn, v, W_ah, b_ah, W_out, b_out):
    f = np.float32
    enc = np.asarray(encoder_out, f)
    word = int(np.asarray(word_input).reshape(-1)[0])
    embedded = np.asarray(emb, f)[word]
    x = np.concatenate([np.asarray(last_context, f)[0], embedded])  # [H+E]
    wih_np, wsm_np = _np_dt(WIH_DT), _np_dt(WSM_DT)
    wo_np, wahc_np, att_np = _np_dt(WO_DT), _np_dt(WAHC_DT), _np_dt(ATT_DT)
    xk = np.ascontiguousarray(x.reshape(KX, 128).T).astype(wih_np)

    h0v = np.asarray(h0, f)[0, 0]
    c0v = np.asarray(c0, f)[0, 0]
    hbias = np.asarray(b_ih, f) + np.asarray(b_hh, f)
    if h0v.any():
        hbias = hbias + np.asarray(W_hh, f) @ h0v

    W_ih = np.asarray(W_ih, f)
    W_attn = np.asarray(W_attn, f)
    Wa_h, Wa_e = W_attn[:, :H], W_attn[:, H:]
    waeT = np.ascontiguousarray(Wa_e.T).astype(att_np)
    W_ah = np.asarray(W_ah, f)
    wahcT = np.ascontiguousarray(W_ah[:, :H].T).astype(wahc_np)  # [ctx, h]
    W_out = np.asarray(W_out, f)
    b_out = np.asarray(b_out, f)
    v = np.asarray(v, f)
    vk = np.ascontiguousarray(v.reshape(KH, 128).T)
    b_attn = np.asarray(b_attn, f)
    battn8 = np.ascontiguousarray(b_attn.reshape(KH, 128).T)     # [128, 8]
    bah8 = np.ascontiguousarray(np.asarray(b_ah, f).reshape(KH, 128).T)

    GORDER = (0, 1, 3, 2)  # i, f, o, g
    in_maps = []
    for k in range(NCORES):
        hs = slice(k * HC, (k + 1) * HC)
        rows = np.concatenate([np.arange(g * H + k * HC, g * H + (k + 1) * HC)
                               for g in GORDER])
        wihT = np.ascontiguousarray(W_ih[rows, :].T).astype(wih_np)
        hbiasr = np.ascontiguousarray(hbias[rows].reshape(1, 512)).astype(wih_np)
        c0_k = np.ascontiguousarray(c0v[hs].reshape(HC, 1))
        # column shards of Wa_h and W_ah[:, H:]: [128(k-chunk), 1024] each
        wattT = np.ascontiguousarray(
            np.concatenate([Wa_h[:, hs].T, W_ah[:, H + k * HC: H + (k + 1) * HC].T],
                           axis=1)).astype(wsm_np)               # [128, 2048]
        encT_k = np.ascontiguousarray(enc[k * SC:(k + 1) * SC, :].T).astype(att_np)
        encN_k = np.ascontiguousarray(enc[k * SC:(k + 1) * SC, :]).astype(att_np)
        wo_pad = np.zeros((VP, H), f)
        wo_pad[:VC] = W_out[k * VC:(k + 1) * VC, :]
        w4 = wo_pad.T.reshape(KH, 128, 32, 128)                  # [ec, p, mc, q]
        w4 = w4.transpose(2, 1, 0, 3)                            # [mc, p, ec, q]
        w4 = w4.reshape(8, 4, 128, KH, 128).transpose(0, 2, 1, 3, 4)
        woutP = np.ascontiguousarray(w4.reshape(8, 128, 4096)).astype(wo_np)
        bo_pad = np.full(VP, NEG_BIG, f)
        bo_pad[:VC] = b_out[k * VC:(k + 1) * VC]
        bout_k = np.ascontiguousarray(bo_pad.reshape(32, 128).T)
        in_maps.append({
            "xk": xk, "hbiasr": hbiasr, "c0k": c0_k, "wihT": wihT,
            "wattT": wattT, "battn8": battn8, "waeT": waeT,
            "encT": encT_k, "encN": encN_k, "vk": vk,
            "wahcT": wahcT, "bah8": bah8, "woutP": woutP, "bout": bout_k,
        })
    return in_maps


def assemble_outputs(results):
    f = np.float32
    ht = np.concatenate([results[k]["hout"].reshape(-1) for k in range(NCORES)])
    c = np.concatenate([results[k]["cout"].reshape(-1) for k in range(NCORES)])
    # httout is the FULL ht_tilda in K-layout [128,8] (identical on all cores)
    htt = results[0]["httout"].T.reshape(-1)
    w = np.concatenate(
        [results[k]["wout"].T.reshape(-1) for k in range(NCORES)])
    out = np.concatenate(
        [results[k]["lpout"].T.reshape(-1)[:VC] for k in range(NCORES)])
    return (out[None, :].astype(f),
            (ht[None, None, :].astype(f), c[None, None, :].astype(f)),
            htt[None, :].astype(f), w[None, :].astype(f))


_cached_nc = None


def kernel(**inputs):
    global _cached_nc
    if _cached_nc is None:
        _cached_nc = build_module(n_iters=1)
    in_maps = shard_inputs(**inputs)
    res = bass_utils.run_bass_kernel_spmd(
        _cached_nc, in_maps, core_ids=list(range(NCORES)))
    return assemble_outputs(res.results)


if __name__ == "__main__":
    import jax
    import reference
    with jax.default_device(jax.devices("cpu")[0]):
        inputs = {k: np.asarray(val) for k, val in reference.setup_inputs().items()}
        expected = jax.tree.map(np.asarray, reference.reference(**inputs))
    actual = kernel(**inputs)
    for (ep, e), (ap_, a) in zip(
            jax.tree_util.tree_leaves_with_path(expected),
            jax.tree_util.tree_leaves_with_path(actual)):
        e = np.asarray(e); a = np.asarray(a)
        rel = np.abs(a - e).max() / (np.abs(e).max() + 1e-12)
        print(f"{jax.tree_util.keystr(ep)}: rel={rel:.3e}")
